# revision 1
# baseline (speedup 1.0000x reference)
"""3-layer GAT (graph attention network) forward pass on 8 Trainium2 cores.

Strategy (graph/data parallel per the destination-node partition):
  - Nodes are partitioned contiguously across 8 cores (12500 each), then
    re-binned within each core into 98 blocks of 128 nodes with balanced
    in-degree (LPT), so every (core, block) has the same padded edge-tile
    count T. The SPMD program is therefore identical on all cores; only the
    data (index arrays) differs.
  - Per layer: each core computes its own chunk of the gather table
    [W@h | el] ([feat2|el2] for the output layer) and AllGathers it, plus a
    core-local er table. Edge phase: per 128-node block, indirect-DMA gather
    of per-edge source rows by src index, per-edge er by dst index, then
    ee = exp(leaky_relu(el_src + er_dst)) and a one-hot segment matmul that
    accumulates [sum(ee*feat) | sum(ee)] over the block's edges in PSUM.
    Softmax normalization (divide by sum(ee)) happens per node afterwards —
    numerically safe here because |e| stays small, so no segment-max shift.
  - Layers 0/1 keep everything transposed ([channel, node]) so BN reduces
    along the free axis and BN+ReLU applies as one activation op with
    per-partition scale/bias; BN makes the +b0/+b1 biases no-ops, so they
    are dropped. BN stats are AllReduced across cores.
  - Layer 2 runs node-major and writes output rows directly.
  - One NEFF per layer: walrus tracks SWDGE DMA-queue completion counts in a
    16-bit ISA field across a whole NEFF (~4096 indirect DMAs max), so the
    three layers are separate programs; h is handed between them by the host.
"""

import heapq
import numpy as np

import concourse.bass as bass
import concourse.bacc as bacc
import concourse.tile as tile
from concourse import mybir, bass_utils

F32 = mybir.dt.float32
I32 = mybir.dt.int32
BF16 = mybir.dt.bfloat16

# Layers 0/1 gather-table rows stored as [Wh bf16 | el f32-bitcast]:
# halves the dominant indirect-gather bytes; attention logits stay f32.
TBL_BF16 = False


class Cfg:
    def __init__(self, n, e, ncores, in_dim=128, hid=16, heads=4, outc=40,
                 neg=0.2, eps=1e-5):
        assert n % ncores == 0
        self.N, self.E, self.NCORES = n, e, ncores
        self.IN, self.HID, self.HEADS, self.OUTC = in_dim, hid, heads, outc
        self.F = heads * hid          # 64
        self.F2 = heads * outc        # 160
        self.P = 128
        self.NPC = n // ncores        # real nodes per core
        self.BLOCKS = (self.NPC + self.P - 1) // self.P
        self.NOWN = self.BLOCKS * self.P   # padded nodes per core
        self.NPAD = ncores * self.NOWN
        self.NEG, self.EPS = neg, eps


CFG = Cfg(100000, 1600000, 8)


# ---------------------------------------------------------------- host prep

def _bin_nodes(cfg, deg):
    """LPT-bin each core's nodes into BLOCKS bins of P slots, balancing
    in-degree sums. Returns newlocal[node] = padded local id on its core."""
    newlocal = np.empty(cfg.N, np.int64)
    # capacity: last bin holds only the remainder so the unfilled (pad)
    # slots are exactly local ids [NPC, NOWN) — the device zeroes that range
    cap = np.full(cfg.BLOCKS, cfg.P, np.int64)
    cap[-1] = cfg.NPC - (cfg.BLOCKS - 1) * cfg.P
    for r in range(cfg.NCORES):
        lo, hi = r * cfg.NPC, (r + 1) * cfg.NPC
        d = deg[lo:hi]
        order = np.argsort(-d, kind="stable")
        heap = [(0, b) for b in range(cfg.BLOCKS)]
        heapq.heapify(heap)
        slots = np.zeros(cfg.BLOCKS, np.int64)
        loc = np.empty(cfg.NPC, np.int64)
        for i in order:
            while True:
                c, b = heapq.heappop(heap)
                if slots[b] < cap[b]:
                    break
            loc[i] = b * cfg.P + slots[b]
            slots[b] += 1
            heapq.heappush(heap, (c + int(d[i]), b))
        newlocal[lo:hi] = loc
    return newlocal


def preprocess(cfg, src, dst):
    """Static graph preprocessing. Returns per-core index arrays + T + perm."""
    src = np.asarray(src, np.int64)
    dst = np.asarray(dst, np.int64)
    deg = np.bincount(dst, minlength=cfg.N)
    newlocal = _bin_nodes(cfg, deg)
    core_of = np.arange(cfg.N) // cfg.NPC
    newglobal = core_of * cfg.NOWN + newlocal

    r_e = dst // cfg.NPC
    dloc = newlocal[dst]
    b_e = dloc // cfg.P
    slot_e = dloc % cfg.P
    s_glob = newglobal[src]

    key = r_e * cfg.BLOCKS + b_e
    order_e = np.argsort(key, kind="stable")
    cnt = np.bincount(key, minlength=cfg.NCORES * cfg.BLOCKS)
    T = int(np.ceil(cnt.max() / cfg.P))
    starts = np.concatenate([[0], np.cumsum(cnt)])

    se = s_glob[order_e]
    sl = slot_e[order_e]
    dl = dloc[order_e]

    TP = T * cfg.P
    srcidx = np.zeros((cfg.NCORES, cfg.BLOCKS, TP), np.int32)
    eridx = np.full((cfg.NCORES, cfg.BLOCKS, TP), cfg.NOWN, np.int32)
    colv = np.full((cfg.NCORES, cfg.BLOCKS, TP), -1.0, np.float32)
    for r in range(cfg.NCORES):
        for b in range(cfg.BLOCKS):
            k = r * cfg.BLOCKS + b
            c0, c1 = starts[k], starts[k + 1]
            n = c1 - c0
            srcidx[r, b, :n] = se[c0:c1]
            eridx[r, b, :n] = dl[c0:c1]
            colv[r, b, :n] = sl[c0:c1]

    # [r, b, T*P] -> [r, b, P, T]: edge k of a block -> tile k//P, partition k%P
    srcidx = np.ascontiguousarray(srcidx.reshape(cfg.NCORES, cfg.BLOCKS, T, cfg.P).transpose(0, 1, 3, 2))
    eridx = np.ascontiguousarray(eridx.reshape(cfg.NCORES, cfg.BLOCKS, T, cfg.P).transpose(0, 1, 3, 2))
    colv = np.ascontiguousarray(colv.reshape(cfg.NCORES, cfg.BLOCKS, T, cfg.P).transpose(0, 1, 3, 2))
    return srcidx, eridx, colv, T, newlocal


def _fold_el(W, a, heads, dph):
    # w[k, h] = sum_d W[k, h*dph+d] * a[h, d]
    return np.einsum("khd,hd->kh", W.reshape(W.shape[0], heads, dph), a).astype(np.float32)


# ---------------------------------------------------------------- program

def build_layer_program(cfg, T, layer):
    P, F, F2, IN = cfg.P, cfg.F, cfg.F2, cfg.IN
    H, D, OC = cfg.HEADS, cfg.HID, cfg.OUTC
    B = cfg.BLOCKS
    fo = F2 if layer == 2 else F
    fin = IN if layer == 0 else F
    nc = bacc.Bacc("TRN2", target_bir_lowering=False, debug=False,
                   num_devices=cfg.NCORES)

    def inp(name, shape, dt=F32):
        return nc.dram_tensor(name, shape, dt, kind="ExternalInput").ap()

    x_in = inp("x_in", [fin, cfg.NOWN])
    srcidx = inp("srcidx", [B, P, T], I32)
    eridx = inp("eridx", [B, P, T], I32)
    colv = inp("colv", [B, P, T])
    Wext_i = inp("Wext", [fin, fo + 8])
    iota_in = inp("iota", [P, P])
    if layer < 2:
        gamma_i = inp("gamma", [F, 1])
        beta_i = inp("beta", [F, 1])
        bsel_in = inp("bsel", [H, F])
        y_out = nc.dram_tensor("y_out", [F, cfg.NOWN], F32,
                               kind="ExternalOutput").ap()
    if layer == 0:
        resW_i = inp("resW", [IN, F])
    if layer == 2:
        resW_i = inp("resW", [F, F2])
        b2bc_in = inp("b2bc", [P, F2])
        out = nc.dram_tensor("out", [cfg.NOWN, F2], F32,
                             kind="ExternalOutput").ap()

    rg = [list(range(cfg.NCORES))]

    with tile.TileContext(nc) as tc:
        with (
            tc.tile_pool(name="big", bufs=1) as bigp,
            tc.tile_pool(name="const", bufs=1) as cons,
            tc.tile_pool(name="work", bufs=3) as wk,
            tc.tile_pool(name="ps", bufs=2, space="PSUM") as ps,
            tc.tile_pool(name="dram", bufs=1, space="DRAM") as dr,
        ):
            # DRAM scratch
            bf = TBL_BF16 and layer < 2
            tdt = BF16 if bf else F32
            fw = fo + 8 if bf else fo + 4  # gather row width (tdt units)
            tbl_own = dr.tile([cfg.NOWN, fw], tdt)
            tbl_full = dr.tile([cfg.NPAD, fw], tdt, addr_space="Shared")
            er_dram = dr.tile([cfg.NOWN + 1, 4], F32)
            if layer < 2:
                stats_in = dr.tile([F, 2], F32)
                stats_out = dr.tile([F, 2], F32, addr_space="Shared")

            # per-layer SBUF state
            xT = bigp.tile([fin, cfg.NOWN], F32, tag="xT")
            nc.sync.dma_start(out=xT[:], in_=x_in[:])
            if layer < 2:
                yT = bigp.tile([F, cfg.NOWN], F32, tag="yT")

            Wext = cons.tile([fin, fo + 8], F32, tag="Wext")
            nc.sync.dma_start(out=Wext[:], in_=Wext_i[:])
            iota_f = cons.tile([P, P], F32, tag="iota_sb")
            nc.sync.dma_start(out=iota_f[:], in_=iota_in[:])
            if layer < 2:
                bsel = cons.tile([H, F], F32, tag="bsel_sb")
                nc.sync.dma_start(out=bsel[:], in_=bsel_in[:])
                gam = cons.tile([F, 1], F32, tag="gam")
                bet = cons.tile([F, 1], F32, tag="bet")
                nc.sync.dma_start(out=gam[:], in_=gamma_i[:])
                nc.sync.dma_start(out=bet[:], in_=beta_i[:])
                stats = cons.tile([F, 2], F32, tag="stats")
                nc.vector.memset(stats[:], 0.0)
            if layer == 0:
                rW = cons.tile([IN, F], F32, tag="rW")
                nc.sync.dma_start(out=rW[:], in_=resW_i[:])
            if layer == 2:
                rW = cons.tile([F, F2], F32, tag="rW")
                nc.sync.dma_start(out=rW[:], in_=resW_i[:])
                b2bc = cons.tile([P, F2], F32, tag="b2bc_sb")
                nc.sync.dma_start(out=b2bc[:], in_=b2bc_in[:])
            padrow = cons.tile([1, 4], F32, tag="padrow")
            nc.vector.memset(padrow[:], -1e38)

            # ---- phase A: gather tables + AllGather ----
            for b in range(B):
                tp = ps.tile([P, fo + 8], F32, tag="tbl")
                nc.tensor.matmul(out=tp[:], lhsT=xT[:, b * P:(b + 1) * P],
                                 rhs=Wext[:], start=True, stop=True)
                if bf:
                    rowb = wk.tile([P, fo + 8], BF16, tag="trow")
                    nc.vector.tensor_copy(out=rowb[:, :fo], in_=tp[:, :fo])
                    nc.vector.tensor_copy(
                        out=rowb[:, fo:fo + 8].bitcast(F32),
                        in_=tp[:, fo:fo + 4])
                    erow = wk.tile([P, 4], F32, tag="erow")
                    nc.vector.tensor_copy(out=erow[:], in_=tp[:, fo + 4:fo + 8])
                    nc.sync.dma_start(out=tbl_own[b * P:(b + 1) * P, :],
                                      in_=rowb[:])
                    nc.sync.dma_start(out=er_dram[b * P:(b + 1) * P, :],
                                      in_=erow[:])
                else:
                    row = wk.tile([P, fo + 8], F32, tag="trow")
                    nc.vector.tensor_copy(out=row[:], in_=tp[:])
                    nc.sync.dma_start(out=tbl_own[b * P:(b + 1) * P, :],
                                      in_=row[:, :fo + 4])
                    nc.sync.dma_start(out=er_dram[b * P:(b + 1) * P, :],
                                      in_=row[:, fo + 4:fo + 8])
            nc.sync.dma_start(out=er_dram[cfg.NOWN:cfg.NOWN + 1, :],
                              in_=padrow[:])
            nc.gpsimd.collective_compute(
                "AllGather", mybir.AluOpType.bypass, replica_groups=rg,
                ins=[tbl_own[:].opt()], outs=[tbl_full[:].opt()])
            # gathers race the collective's remote writes without this:
            tc.strict_bb_all_engine_barrier()

            # ---- phase B: edge aggregation per block ----
            fo4 = fo + 4
            dph = OC if layer == 2 else D
            for b in range(B):
                sidx = wk.tile([P, T], I32, tag="sidx")
                eidx = wk.tile([P, T], I32, tag="eidx")
                colf = wk.tile([P, T], F32, tag="colf")
                nc.sync.dma_start(out=sidx[:], in_=srcidx[b])
                nc.sync.dma_start(out=eidx[:], in_=eridx[b])
                nc.sync.dma_start(out=colf[:], in_=colv[b])

                G = wk.tile([P, T * fw], tdt, tag="G")
                R = wk.tile([P, T * 4], F32, tag="R")
                for t in range(T):
                    nc.gpsimd.indirect_dma_start(
                        out=G[:, t * fw:(t + 1) * fw], out_offset=None,
                        in_=tbl_full[:],
                        in_offset=bass.IndirectOffsetOnAxis(
                            ap=sidx[:, t:t + 1], axis=0))
                    nc.gpsimd.indirect_dma_start(
                        out=R[:, t * 4:(t + 1) * 4], out_offset=None,
                        in_=er_dram[:],
                        in_offset=bass.IndirectOffsetOnAxis(
                            ap=eidx[:, t:t + 1], axis=0))

                G3 = G[:].rearrange("p (t f) -> p t f", t=T)
                el_view = (G3[:, :, fo:fo + 8].bitcast(F32) if bf
                           else G3[:, :, fo:fo4])
                ee = wk.tile([P, T * H], F32, tag="ee")
                ee3 = ee[:].rearrange("p (t h) -> p t h", t=T)
                nc.vector.tensor_tensor(
                    out=ee3, in0=el_view,
                    in1=R[:].rearrange("p (t h) -> p t h", t=T),
                    op=mybir.AluOpType.add)
                esc = wk.tile([P, T * H], F32, tag="esc")
                nc.vector.tensor_scalar_mul(out=esc[:], in0=ee[:],
                                            scalar1=cfg.NEG)
                nc.vector.tensor_tensor(out=ee[:], in0=ee[:], in1=esc[:],
                                        op=mybir.AluOpType.max)
                nc.scalar.activation(out=ee[:], in_=ee[:],
                                     func=mybir.ActivationFunctionType.Exp)

                O = wk.tile([P, T * P], F32, tag="O")
                nc.vector.tensor_tensor(
                    out=O[:].rearrange("p (t c) -> p t c", t=T),
                    in0=iota_f[:].unsqueeze(1).to_broadcast([P, T, P]),
                    in1=colf[:].unsqueeze(2).to_broadcast([P, T, P]),
                    op=mybir.AluOpType.is_equal)

                V = wk.tile([P, T * fo4], F32, tag="V")
                V3 = V[:].rearrange("p (t f) -> p t f", t=T)
                nc.vector.tensor_tensor(
                    out=V3[:, :, :fo].rearrange("p t (h d) -> p t h d", h=H),
                    in0=G3[:, :, :fo].rearrange("p t (h d) -> p t h d", h=H),
                    in1=ee3.unsqueeze(3).to_broadcast([P, T, H, dph]),
                    op=mybir.AluOpType.mult)
                nc.vector.tensor_copy(out=V3[:, :, fo:fo4], in_=ee3)

                if layer < 2:
                    # transposed accumulate: acc[fo+4, dst] += V_t.T @ O_t
                    acc = ps.tile([F + 4, P], F32, tag="acc")
                    for t in range(T):
                        nc.tensor.matmul(
                            out=acc[:], lhsT=V[:, t * fo4:(t + 1) * fo4],
                            rhs=O[:, t * P:(t + 1) * P],
                            start=(t == 0), stop=(t == T - 1))
                    dmax = wk.tile([H, P], F32, tag="dmax")
                    nc.vector.tensor_scalar_max(out=dmax[:],
                                                in0=acc[F:F + 4, :],
                                                scalar1=1e-16)
                    rec = wk.tile([H, P], F32, tag="rec")
                    nc.vector.reciprocal(out=rec[:], in_=dmax[:])
                    recb_ps = ps.tile([F, P], F32, tag="aux")
                    nc.tensor.matmul(out=recb_ps[:], lhsT=bsel[:],
                                     rhs=rec[:], start=True, stop=True)
                    recb = wk.tile([F, P], F32, tag="recb")
                    nc.vector.tensor_copy(out=recb[:], in_=recb_ps[:])

                    hsl = yT[:, b * P:(b + 1) * P]
                    nc.vector.tensor_tensor(out=hsl, in0=acc[:F, :],
                                            in1=recb[:],
                                            op=mybir.AluOpType.mult)
                    if layer == 0:
                        res_ps = ps.tile([F, P], F32, tag="aux")
                        nc.tensor.matmul(out=res_ps[:], lhsT=rW[:],
                                         rhs=xT[:, b * P:(b + 1) * P],
                                         start=True, stop=True)
                        nc.vector.tensor_tensor(out=hsl, in0=hsl,
                                                in1=res_ps[:],
                                                op=mybir.AluOpType.add)
                    else:
                        hold = xT[:, b * P:(b + 1) * P]
                        nc.vector.tensor_tensor(out=hsl, in0=hsl, in1=hold,
                                                op=mybir.AluOpType.add)
                        nc.vector.tensor_tensor(out=hsl, in0=hsl, in1=hold,
                                                op=mybir.AluOpType.add)

                    red = wk.tile([F, 1], F32, tag="red")
                    nc.vector.reduce_sum(out=red[:], in_=hsl,
                                         axis=mybir.AxisListType.X)
                    nc.vector.tensor_tensor(out=stats[:, 0:1],
                                            in0=stats[:, 0:1], in1=red[:],
                                            op=mybir.AluOpType.add)
                    sq = wk.tile([F, P], F32, tag="sq")
                    nc.scalar.activation(
                        out=sq[:], in_=hsl,
                        func=mybir.ActivationFunctionType.Square)
                    nc.vector.reduce_sum(out=red[:], in_=sq[:],
                                         axis=mybir.AxisListType.X)
                    nc.vector.tensor_tensor(out=stats[:, 1:2],
                                            in0=stats[:, 1:2], in1=red[:],
                                            op=mybir.AluOpType.add)
                else:
                    # node-major accumulate: acc[dst, fo+4] += O_t.T @ V_t
                    acc = ps.tile([P, F2 + 4], F32, tag="acc")
                    for t in range(T):
                        nc.tensor.matmul(
                            out=acc[:], lhsT=O[:, t * P:(t + 1) * P],
                            rhs=V[:, t * fo4:(t + 1) * fo4],
                            start=(t == 0), stop=(t == T - 1))
                    dmax = wk.tile([P, H], F32, tag="dmax")
                    nc.vector.tensor_scalar_max(out=dmax[:],
                                                in0=acc[:, F2:F2 + 4],
                                                scalar1=1e-16)
                    rec = wk.tile([P, H], F32, tag="rec")
                    nc.vector.reciprocal(out=rec[:], in_=dmax[:])
                    rst = wk.tile([P, F2], F32, tag="rst")
                    nc.vector.tensor_tensor(
                        out=rst[:].rearrange("p (h o) -> p h o", h=H),
                        in0=acc[:, :F2].rearrange("p (h o) -> p h o", h=H),
                        in1=rec[:].unsqueeze(2).to_broadcast([P, H, OC]),
                        op=mybir.AluOpType.mult)
                    res_ps = ps.tile([P, F2], F32, tag="aux")
                    nc.tensor.matmul(out=res_ps[:],
                                     lhsT=xT[:, b * P:(b + 1) * P],
                                     rhs=rW[:], start=True, stop=True)
                    nc.vector.tensor_tensor(out=rst[:], in0=rst[:],
                                            in1=res_ps[:],
                                            op=mybir.AluOpType.add)
                    nc.vector.tensor_tensor(out=rst[:], in0=rst[:],
                                            in1=b2bc[:],
                                            op=mybir.AluOpType.add)
                    nc.sync.dma_start(out=out[b * P:(b + 1) * P, :],
                                      in_=rst[:])

            # ---- phase C: BN (layers 0/1) + writeback ----
            if layer < 2:
                nc.sync.dma_start(out=stats_in[:], in_=stats[:])
                nc.gpsimd.collective_compute(
                    "AllReduce", mybir.AluOpType.add, replica_groups=rg,
                    ins=[stats_in[:].opt()], outs=[stats_out[:].opt()])
                tc.strict_bb_all_engine_barrier()
                ssb = cons.tile([F, 2], F32, tag="ssb")
                nc.sync.dma_start(out=ssb[:], in_=stats_out[:])
                mu = cons.tile([F, 1], F32, tag="mu")
                tmp1 = cons.tile([F, 1], F32, tag="tmp1")
                scl = cons.tile([F, 1], F32, tag="scl")
                bia = cons.tile([F, 1], F32, tag="bia")
                musq = cons.tile([F, 1], F32, tag="musq")
                invn = 1.0 / cfg.N
                nc.vector.tensor_scalar_mul(out=mu[:], in0=ssb[:, 0:1],
                                            scalar1=invn)
                nc.vector.tensor_scalar_mul(out=tmp1[:], in0=ssb[:, 1:2],
                                            scalar1=invn)
                nc.scalar.activation(out=musq[:], in_=mu[:],
                                     func=mybir.ActivationFunctionType.Square)
                nc.vector.tensor_tensor(out=tmp1[:], in0=tmp1[:],
                                        in1=musq[:],
                                        op=mybir.AluOpType.subtract)
                nc.vector.tensor_scalar_add(out=tmp1[:], in0=tmp1[:],
                                            scalar1=cfg.EPS)
                nc.scalar.activation(out=tmp1[:], in_=tmp1[:],
                                     func=mybir.ActivationFunctionType.Sqrt)
                nc.vector.reciprocal(out=tmp1[:], in_=tmp1[:])
                nc.vector.tensor_tensor(out=scl[:], in0=tmp1[:], in1=gam[:],
                                        op=mybir.AluOpType.mult)
                nc.vector.tensor_tensor(out=tmp1[:], in0=mu[:], in1=scl[:],
                                        op=mybir.AluOpType.mult)
                nc.vector.tensor_tensor(out=bia[:], in0=bet[:], in1=tmp1[:],
                                        op=mybir.AluOpType.subtract)
                nc.scalar.activation(out=yT[:, :], in_=yT[:, :],
                                     func=mybir.ActivationFunctionType.Relu,
                                     scale=scl[:, 0:1], bias=bia[:, 0:1])
                if cfg.NPC < cfg.NOWN:
                    nc.vector.memset(yT[:, cfg.NPC:cfg.NOWN], 0.0)
                nc.sync.dma_start(out=y_out[:], in_=yT[:])

    nc.compile()
    return nc


# ---------------------------------------------------------------- host glue

def make_in_maps(cfg, inputs, srcidx, eridx, colv, newlocal):
    feat = np.asarray(inputs["feat"], np.float32)
    H, D, OC = cfg.HEADS, cfg.HID, cfg.OUTC
    W0 = np.asarray(inputs["W0"], np.float32)
    W1 = np.asarray(inputs["W1"], np.float32)
    W2 = np.asarray(inputs["W2"], np.float32)
    W0ext = np.concatenate([W0, _fold_el(W0, np.asarray(inputs["al0"]), H, D),
                            _fold_el(W0, np.asarray(inputs["ar0"]), H, D)], axis=1)
    W1ext = np.concatenate([W1, _fold_el(W1, np.asarray(inputs["al1"]), H, D),
                            _fold_el(W1, np.asarray(inputs["ar1"]), H, D)], axis=1)
    W2ext = np.concatenate([W2, _fold_el(W2, np.asarray(inputs["al2"]), H, OC),
                            _fold_el(W2, np.asarray(inputs["ar2"]), H, OC)], axis=1)

    common = {
        "bsel": np.repeat(np.eye(cfg.HEADS, dtype=np.float32), cfg.HID, axis=1),
        "iota": np.tile(np.arange(cfg.P, dtype=np.float32)[None, :], (cfg.P, 1)),
    }
    layer_maps = [[], [], []]
    for r in range(cfg.NCORES):
        ids = np.arange(r * cfg.NPC, (r + 1) * cfg.NPC)
        fp = np.zeros((cfg.NOWN, cfg.IN), np.float32)
        fp[newlocal[ids]] = feat[ids]
        idx = {"srcidx": srcidx[r], "eridx": eridx[r], "colv": colv[r]}
        layer_maps[0].append({
            "x_in": np.ascontiguousarray(fp.T), "Wext": W0ext,
            "resW": np.asarray(inputs["resW0"], np.float32),
            "gamma": np.asarray(inputs["gamma0"], np.float32).reshape(cfg.F, 1),
            "beta": np.asarray(inputs["beta0"], np.float32).reshape(cfg.F, 1),
            **idx, **common,
        })
        layer_maps[1].append({
            "Wext": W1ext,
            "gamma": np.asarray(inputs["gamma1"], np.float32).reshape(cfg.F, 1),
            "beta": np.asarray(inputs["beta1"], np.float32).reshape(cfg.F, 1),
            **idx, **common,
        })
        layer_maps[2].append({
            "Wext": W2ext,
            "resW": np.asarray(inputs["resW2"], np.float32),
            "b2bc": np.tile(np.asarray(inputs["b2"], np.float32).reshape(1, cfg.F2),
                            (cfg.P, 1)),
            "iota": common["iota"],
            **idx,
        })
    return layer_maps


def assemble_output(cfg, results, newlocal):
    out = np.empty((cfg.N, cfg.F2), np.float32)
    for r in range(cfg.NCORES):
        ids = np.arange(r * cfg.NPC, (r + 1) * cfg.NPC)
        out[ids] = results[r]["out"][newlocal[ids]]
    return out


_PROG_CACHE = {}


def get_program(cfg, T, layer):
    key = (cfg.N, cfg.E, T, layer, TBL_BF16)
    if key not in _PROG_CACHE:
        _PROG_CACHE[key] = build_layer_program(cfg, T, layer)
    return _PROG_CACHE[key]


def run(inputs, trace=False, trace_cores=None):
    cfg = CFG
    src = np.asarray(inputs["src"])
    dst = np.asarray(inputs["dst"])
    srcidx, eridx, colv, T, newlocal = preprocess(cfg, src, dst)
    layer_maps = make_in_maps(cfg, inputs, srcidx, eridx, colv, newlocal)
    cores = list(range(cfg.NCORES))
    total_ns = 0
    layer_res = []
    for layer in range(3):
        nc = get_program(cfg, T, layer)
        res = bass_utils.run_bass_kernel_spmd(
            nc, layer_maps[layer], core_ids=cores,
            trace=trace, trace_cores=trace_cores)
        layer_res.append(res)
        if res.exec_time_ns:
            total_ns += res.exec_time_ns
        if layer < 2:
            for r in range(cfg.NCORES):
                layer_maps[layer + 1][r]["x_in"] = res.results[r]["y_out"]
    out = assemble_output(cfg, layer_res[2].results, newlocal)
    return out, (total_ns, layer_res)


def kernel(**inputs) -> np.ndarray:
    return run(inputs)[0]

